# revision 4
# baseline (speedup 1.0000x reference)
"""Bezier curve Gaussian rasterization on 8 Trainium2 NeuronCores.

Problem: curves [8,4,2] -> raster [512,512] where
    raster.T[b,a] = sum_s Ey[b,s] * Ex[a,s]
    Ex[a,s] = exp(-5000*(x_s - a/512)^2),  x_s = cubic Bezier samples,
    T = 8 curves x 128 t-samples = 1024.

Strategy (no collectives -- their ~10us floor dwarfs this kernel):
shard OUTPUT ROWS b across the 8 cores. Core k computes
out[64k:64k+64, :] = EyT_k.T @ ExT  with the s-contraction (1024) done
as 8 accumulating PE matmuls. Each core computes the full ExT
(s on partitions, 8 tiles of [128, 512+64]) via DVE sub/square + ACT exp,
with the Bezier sampling itself done as a tiny PE matmul against a
baked Bernstein basis.

kernel(curves) -> np.ndarray [512,512] float32.
"""
import sys
import types

import numpy as np

RES = 512
STEPS = 128
N_CURVES = 8
N_CORES = 8
BROWS = RES // N_CORES  # 64 output rows per core
SIGMA = 0.01
# exp scale in pixel units: -(1/(2 sigma^2)) / RES^2
EXP_SCALE = -1.0 / (2.0 * SIGMA * SIGMA) / (RES * RES)

_CACHE = {}


def _install_ntff_hook():
    """Provide antenv.axon_hooks (missing in this image) so NTFF
    profiling via run_bass_kernel_spmd(trace=True) works."""
    try:
        import antenv
    except ImportError:
        return
    if "antenv.axon_hooks" in sys.modules:
        return
    mod = types.ModuleType("antenv.axon_hooks")
    _state = {"hook": None}
    mod.set_axon_ntff_profile_hook = lambda h: _state.__setitem__("hook", h)
    mod.get_axon_ntff_profile_hook = lambda: _state["hook"]
    sys.modules["antenv.axon_hooks"] = mod
    antenv.axon_hooks = mod
    try:
        from trn_agent_boot.trn_boot import _ntff_profile_via_ctypes

        hook = _ntff_profile_via_ctypes("/opt/axon/libaxon_pjrt.so")
        if hook is not None:
            mod.set_axon_ntff_profile_hook(hook)
    except Exception:
        pass


def _bernstein_basis() -> np.ndarray:
    """bt [4, 128]: bt[j, p] = B_j(t_p), t = linspace(0,1,128) fp32."""
    t = np.linspace(0.0, 1.0, STEPS, dtype=np.float32).astype(np.float64)
    u = 1.0 - t
    bt = np.stack([u**3, 3 * t * u**2, 3 * t**2 * u, t**3])
    return bt.astype(np.float32)


def build_bass():
    import concourse.bass as bass
    import concourse.tile as tile
    from concourse import bacc, mybir

    W = RES + BROWS  # 576: per-tile width = 512 (a / Ex) + 64 (b-slice / Ey)

    nc = bacc.Bacc("TRN2", target_bir_lowering=False, debug=False, num_devices=N_CORES)
    cv = nc.dram_tensor("cv", [4, 2 * N_CURVES], mybir.dt.float32, kind="ExternalInput").ap()
    bt = nc.dram_tensor("bt", [4, STEPS], mybir.dt.float32, kind="ExternalInput").ap()
    iaa = nc.dram_tensor("iaa", [STEPS, W], mybir.dt.float32, kind="ExternalInput").ap()
    out = nc.dram_tensor("out", [BROWS, RES], mybir.dt.float32, kind="ExternalOutput").ap()

    f32 = mybir.dt.float32
    f32r = mybir.dt.float32r

    with tile.TileContext(nc) as tc:
        with (
            tc.tile_pool(name="const", bufs=1) as cpool,
            tc.tile_pool(name="d", bufs=2) as dpool,
            tc.tile_pool(name="e", bufs=4) as epool,
            tc.tile_pool(name="res", bufs=1) as rpool,
            tc.tile_pool(name="psum", bufs=1, space="PSUM") as ppool,
            tc.tile_pool(name="psum_out", bufs=1, space="PSUM") as opool,
        ):
            cv_sb = cpool.tile([4, 2 * N_CURVES], f32)
            nc.sync.dma_start(out=cv_sb[:], in_=cv[:])
            bt_sb = cpool.tile([4, STEPS], f32)
            nc.sync.dma_start(out=bt_sb[:], in_=bt[:])
            iaa_sb = cpool.tile([STEPS, W], f32)
            nc.sync.dma_start(out=iaa_sb[:], in_=iaa[:])

            # Bezier sampling: xy[p, 2i] = 512*x_i(t_p); xy[p, 2i+1] = 512*y_i(t_p) - 64k
            psum_xy = ppool.tile([STEPS, 2 * N_CURVES], f32)
            nc.tensor.matmul(psum_xy[:], lhsT=bt_sb[:], rhs=cv_sb[:], start=True, stop=True)
            xy_sb = cpool.tile([STEPS, 2 * N_CURVES], f32)
            nc.vector.tensor_copy(out=xy_sb[:], in_=psum_xy[:])

            psum_out = opool.tile([BROWS, RES], f32)

            e_pairs = []
            for pair in range(4):
                d = dpool.tile([STEPS, 2 * W], f32)
                for h in range(2):
                    j = 2 * pair + h
                    off = h * W
                    # d[:, off:off+512] = a - 512*x_j(t)   (per-partition scalar)
                    nc.vector.tensor_scalar_sub(
                        out=d[:, off : off + RES],
                        in0=iaa_sb[:, 0:RES],
                        scalar1=xy_sb[:, 2 * j : 2 * j + 1],
                    )
                    # d[:, off+512:off+576] = r - (512*y_j(t) - 64k)
                    nc.vector.tensor_scalar_sub(
                        out=d[:, off + RES : off + W],
                        in0=iaa_sb[:, RES:W],
                        scalar1=xy_sb[:, 2 * j + 1 : 2 * j + 2],
                    )
                nc.vector.tensor_mul(out=d[:], in0=d[:], in1=d[:])
                e = epool.tile([STEPS, 2 * W], f32r)
                nc.scalar.activation(e[:], d[:], mybir.ActivationFunctionType.Exp, scale=EXP_SCALE)
                e_pairs.append(e)

            for j in range(2 * N_CURVES // 2):
                e = e_pairs[j // 2]
                off = (j % 2) * W
                nc.tensor.matmul(
                    psum_out[:],
                    lhsT=e[:, off + RES : off + W],
                    rhs=e[:, off : off + RES],
                    start=(j == 0),
                    stop=(j == 7),
                )

            res_sb = rpool.tile([BROWS, RES], f32)
            nc.vector.tensor_copy(out=res_sb[:], in_=psum_out[:])
            nc.sync.dma_start(out=out[:], in_=res_sb[:])

    nc.compile()
    return nc


def _make_inputs(curves: np.ndarray):
    """Per-core input maps."""
    bt = _bernstein_basis()
    iaa = np.zeros((STEPS, RES + BROWS), dtype=np.float32)
    iaa[:, :RES] = np.arange(RES, dtype=np.float32)[None, :]
    iaa[:, RES:] = np.arange(BROWS, dtype=np.float32)[None, :]

    xs = (RES * curves[:, :, 0]).astype(np.float32)  # [8,4] = 512*x control pts
    ys = (RES * curves[:, :, 1]).astype(np.float32)

    in_maps = []
    for k in range(N_CORES):
        cvk = np.empty((4, 2 * N_CURVES), dtype=np.float32)
        cvk[:, 0::2] = xs.T
        cvk[:, 1::2] = ys.T - np.float32(BROWS * k)
        in_maps.append({"cv": cvk, "bt": bt, "iaa": iaa})
    return in_maps


def kernel(curves: np.ndarray, trace: bool = False, tmpdir: str | None = None):
    _install_ntff_hook()
    from concourse.bass_utils import run_bass_kernel_spmd

    if "nc" not in _CACHE:
        _CACHE["nc"] = build_bass()
    nc = _CACHE["nc"]

    in_maps = _make_inputs(np.asarray(curves, dtype=np.float32))
    kw = {}
    if trace:
        import concourse.bass_utils as bu

        bu.upload_artifacts = lambda d: d  # no bucket in this container
        kw = {"trace": True, "tmpdir": tmpdir}
    res = run_bass_kernel_spmd(nc, in_maps, core_ids=list(range(N_CORES)), **kw)

    full = np.concatenate([res.results[k]["out"] for k in range(N_CORES)], axis=0)
    if trace:
        return full, res
    return full


# revision 5
# speedup vs baseline: 1.1676x; 1.1676x over previous
"""Bezier curve Gaussian rasterization on 8 Trainium2 NeuronCores.

Problem: curves [8,4,2] -> raster [512,512] where
    out[b,a] = sum_s Ey[b,s] * Ex[a,s]
    Ex[a,s] = exp(-5000*(x_s - a/512)^2),  x_s = cubic Bezier samples,
    T = 8 curves x 128 t-samples = 1024.

Strategy (no collectives -- their ~10us floor dwarfs this kernel):
shard OUTPUT ROWS b across the 8 cores. Core k computes
out[64k:64k+64, :] with the s-contraction (1024) done as 8 accumulating
float32r PE matmuls. Each core computes the full ExT (s on partitions,
8 tiles of [128, 512]) plus its own 64-wide Ey slice:
  d^2 via a custom DVE op sq(in0 - s0) (fuses sub+square, one op),
  exp via ACT, Bezier sampling via a tiny PE matmul against a baked
  Bernstein basis.

kernel(curves) -> np.ndarray [512,512] float32.
"""
import sys
import types

import numpy as np

RES = 512
STEPS = 128
N_CURVES = 8
N_CORES = 8
BROWS = RES // N_CORES  # 64 output rows per core
W = RES + BROWS  # 576 = per-tile width (x part | y part)
SIGMA = 0.01
# exp scale in pixel units: -(1/(2 sigma^2)) / RES^2
EXP_SCALE = -1.0 / (2.0 * SIGMA * SIGMA) / (RES * RES)

_CACHE = {}


def _install_ntff_hook():
    """Provide antenv.axon_hooks (missing in this image) so NTFF
    profiling via run_bass_kernel_spmd(trace=True) works."""
    try:
        import antenv
    except ImportError:
        return
    if "antenv.axon_hooks" in sys.modules:
        return
    mod = types.ModuleType("antenv.axon_hooks")
    _state = {"hook": None}
    mod.set_axon_ntff_profile_hook = lambda h: _state.__setitem__("hook", h)
    mod.get_axon_ntff_profile_hook = lambda: _state["hook"]
    sys.modules["antenv.axon_hooks"] = mod
    antenv.axon_hooks = mod
    try:
        from trn_agent_boot.trn_boot import _ntff_profile_via_ctypes

        hook = _ntff_profile_via_ctypes("/opt/axon/libaxon_pjrt.so")
        if hook is not None:
            mod.set_axon_ntff_profile_hook(hook)
    except Exception:
        pass


def _get_sqdiff():
    """Register (once) a custom DVE op: out = (in0 - s0)^2.

    Fuses the grid-minus-center subtract and the square into a single
    Vector-engine instruction; the table is generated per-NEFF so no
    firmware change is involved.
    """
    if "sqdiff" in _CACHE:
        return _CACHE["sqdiff"]
    from concourse import dve_ops
    from concourse.dve_spec import Spec, Src0, C0, sq, lower, _has_src1
    from concourse.dve_uop import DveOpSpec

    name = "SQDIFF_ANT"
    spec = Spec(
        body=sq(Src0 - C0),
        reference=lambda in0, in1, s0, s1, imm2: (in0.astype(np.float32) - s0) ** 2,
    )
    row = dve_ops._CUSTOM_DVE_ROW_BASE + len(dve_ops.OPS)
    assert row < 0x20
    dve_ops._SUB_OPCODE_FOR_NAME[name] = row
    # Pin the sha of our own lowering output (computed now, same process).
    shas = {}
    for ver in ("v3", "v4"):
        try:
            s = DveOpSpec(name=name, opcode=row, uops=lower(spec, ver=ver),
                          rd1_en=_has_src1(spec))
            shas[ver] = s.sha(ver)
        except Exception:
            pass
    op = dve_ops.DveOp(name, spec, subdim=False, uops_sha=shas)
    dve_ops.OPS.append(op)
    dve_ops.CUSTOM_DVE_SPECS[name] = spec
    _CACHE["sqdiff"] = op
    return op


def _bernstein_basis() -> np.ndarray:
    """bt [4, 128]: bt[j, p] = B_j(t_p), t = linspace(0,1,128) fp32."""
    t = np.linspace(0.0, 1.0, STEPS, dtype=np.float32).astype(np.float64)
    u = 1.0 - t
    bt = np.stack([u**3, 3 * t * u**2, 3 * t**2 * u, t**3])
    return bt.astype(np.float32)


def build_bass():
    import concourse.bass as bass
    import concourse.tile as tile
    from concourse import bacc, mybir

    sqdiff = _get_sqdiff()

    nc = bacc.Bacc("TRN2", target_bir_lowering=False, debug=False, num_devices=N_CORES)
    cv = nc.dram_tensor("cv", [4, 2 * N_CURVES], mybir.dt.float32, kind="ExternalInput").ap()
    bt = nc.dram_tensor("bt", [4, STEPS], mybir.dt.float32, kind="ExternalInput").ap()
    iaa = nc.dram_tensor("iaa", [STEPS, W], mybir.dt.float32, kind="ExternalInput").ap()
    out = nc.dram_tensor("out", [BROWS, RES], mybir.dt.float32, kind="ExternalOutput").ap()

    f32 = mybir.dt.float32
    f32r = mybir.dt.float32r
    Exp = mybir.ActivationFunctionType.Exp
    Square = mybir.ActivationFunctionType.Square

    with tile.TileContext(nc) as tc:
        with (
            tc.tile_pool(name="const", bufs=1) as cpool,
            tc.tile_pool(name="d", bufs=2) as dpool,
            tc.tile_pool(name="e", bufs=4) as epool,
            tc.tile_pool(name="res", bufs=1) as rpool,
            tc.tile_pool(name="psum", bufs=1, space="PSUM") as ppool,
            tc.tile_pool(name="psum_out", bufs=1, space="PSUM") as opool,
        ):
            # small inputs on the gpsimd DMA queue, big const on sync's --
            # they proceed in parallel.
            cv_sb = cpool.tile([4, 2 * N_CURVES], f32)
            nc.gpsimd.dma_start(out=cv_sb[:], in_=cv[:])
            bt_sb = cpool.tile([4, STEPS], f32)
            nc.gpsimd.dma_start(out=bt_sb[:], in_=bt[:])
            iaa_sb = cpool.tile([STEPS, W], f32)
            nc.sync.dma_start(out=iaa_sb[:], in_=iaa[:])

            # Bezier sampling: xy[p, 2i] = 512*x_i(t_p); xy[p,2i+1] = 512*y_i(t_p) - 64k
            psum_xy = ppool.tile([STEPS, 2 * N_CURVES], f32)
            nc.tensor.matmul(psum_xy[:], lhsT=bt_sb[:], rhs=cv_sb[:], start=True, stop=True)
            xy_sb = cpool.tile([STEPS, 2 * N_CURVES], f32)
            nc.vector.tensor_copy(out=xy_sb[:], in_=psum_xy[:])

            psum_out = opool.tile([BROWS, RES], f32)

            e_pairs = []
            for pair in range(4):
                d = dpool.tile([STEPS, 2 * W], f32)
                for h in range(2):
                    j = 2 * pair + h
                    off = h * W
                    if j == 0:
                        # balance: ACT does tile 0's x-part as Square(-a + x)
                        nc.scalar.activation(
                            d[:, off : off + RES], iaa_sb[:, 0:RES], Square,
                            bias=xy_sb[:, 0:1], scale=-1.0,
                        )
                    else:
                        nc.vector._custom_dve(
                            sqdiff,
                            out=d[:, off : off + RES],
                            in0=iaa_sb[:, 0:RES],
                            s0=xy_sb[:, 2 * j : 2 * j + 1],
                        )
                    nc.vector._custom_dve(
                        sqdiff,
                        out=d[:, off + RES : off + W],
                        in0=iaa_sb[:, RES:W],
                        s0=xy_sb[:, 2 * j + 1 : 2 * j + 2],
                    )
                e = epool.tile([STEPS, 2 * W], f32r)
                nc.scalar.activation(e[:], d[:], Exp, scale=EXP_SCALE)
                e_pairs.append(e)

            for j in range(N_CURVES):
                e = e_pairs[j // 2]
                off = (j % 2) * W
                nc.tensor.matmul(
                    psum_out[:],
                    lhsT=e[:, off + RES : off + W],
                    rhs=e[:, off : off + RES],
                    start=(j == 0),
                    stop=(j == 7),
                )

            res_sb = rpool.tile([BROWS, RES], f32)
            nc.scalar.copy(out=res_sb[:], in_=psum_out[:])
            nc.sync.dma_start(out=out[:], in_=res_sb[:])

    nc.compile()
    return nc


def _make_inputs(curves: np.ndarray):
    """Per-core input maps."""
    bt = _bernstein_basis()
    iaa = np.zeros((STEPS, W), dtype=np.float32)
    iaa[:, :RES] = np.arange(RES, dtype=np.float32)[None, :]
    iaa[:, RES:] = np.arange(BROWS, dtype=np.float32)[None, :]

    xs = (RES * curves[:, :, 0]).astype(np.float32)  # [8,4] = 512*x control pts
    ys = (RES * curves[:, :, 1]).astype(np.float32)

    in_maps = []
    for k in range(N_CORES):
        cvk = np.empty((4, 2 * N_CURVES), dtype=np.float32)
        cvk[:, 0::2] = xs.T
        cvk[:, 1::2] = ys.T - np.float32(BROWS * k)
        in_maps.append({"cv": cvk, "bt": bt, "iaa": iaa})
    return in_maps


def kernel(curves: np.ndarray, trace: bool = False, tmpdir: str | None = None):
    _install_ntff_hook()
    from concourse.bass_utils import run_bass_kernel_spmd

    if "nc" not in _CACHE:
        _CACHE["nc"] = build_bass()
    nc = _CACHE["nc"]

    in_maps = _make_inputs(np.asarray(curves, dtype=np.float32))
    kw = {}
    if trace:
        import concourse.bass_utils as bu

        bu.upload_artifacts = lambda d: d  # no bucket in this container
        kw = {"trace": True, "tmpdir": tmpdir}
    res = run_bass_kernel_spmd(nc, in_maps, core_ids=list(range(N_CORES)), **kw)

    full = np.concatenate([res.results[k]["out"] for k in range(N_CORES)], axis=0)
    if trace:
        return full, res
    return full


# revision 8
# speedup vs baseline: 1.2375x; 1.0598x over previous
"""Bezier curve Gaussian rasterization on 8 Trainium2 NeuronCores.

Problem: curves [8,4,2] -> raster [512,512] where
    out[b,a] = sum_s Ey[b,s] * Ex[a,s]
    Ex[a,s] = exp(-5000*(x_s - a/512)^2),  x_s = cubic Bezier samples,
    T = 8 curves x 128 t-samples = 1024.

Strategy (no collectives -- their ~10us floor dwarfs this kernel):
shard OUTPUT ROWS b across the 8 cores. Core k computes
out[64k:64k+64, :] with the s-contraction (1024) done as 8 accumulating
float32r PE matmuls. Each core computes the full ExT (s on partitions,
8 tiles of [128, 512]) plus its own 64-wide Ey slice:
  d^2 via a custom DVE op sq(in0 - s0) (fuses sub+square, one op),
  exp via ACT, Bezier sampling via a tiny PE matmul against a baked
  Bernstein basis.

kernel(curves) -> np.ndarray [512,512] float32.
"""
import sys
import types

import numpy as np

RES = 512
STEPS = 128
N_CURVES = 8
N_CORES = 8
BROWS = RES // N_CORES  # 64 output rows per core
W = RES + BROWS  # 576 = per-tile width (x part | y part)
SIGMA = 0.01
# exp scale in pixel units: -(1/(2 sigma^2)) / RES^2
EXP_SCALE = -1.0 / (2.0 * SIGMA * SIGMA) / (RES * RES)

_CACHE = {}


def _install_ntff_hook():
    """Provide antenv.axon_hooks (missing in this image) so NTFF
    profiling via run_bass_kernel_spmd(trace=True) works."""
    try:
        import antenv
    except ImportError:
        return
    if "antenv.axon_hooks" in sys.modules:
        return
    mod = types.ModuleType("antenv.axon_hooks")
    _state = {"hook": None}
    mod.set_axon_ntff_profile_hook = lambda h: _state.__setitem__("hook", h)
    mod.get_axon_ntff_profile_hook = lambda: _state["hook"]
    sys.modules["antenv.axon_hooks"] = mod
    antenv.axon_hooks = mod
    try:
        from trn_agent_boot.trn_boot import _ntff_profile_via_ctypes

        hook = _ntff_profile_via_ctypes("/opt/axon/libaxon_pjrt.so")
        if hook is not None:
            mod.set_axon_ntff_profile_hook(hook)
    except Exception:
        pass


def _get_sqdiff():
    """Register (once) a custom DVE op: out = (in0 - s0)^2.

    Fuses the grid-minus-center subtract and the square into a single
    Vector-engine instruction; the table is generated per-NEFF so no
    firmware change is involved.
    """
    if "sqdiff" in _CACHE:
        return _CACHE["sqdiff"]
    from concourse import dve_ops
    from concourse.dve_spec import Spec, Src0, C0, sq, lower, _has_src1
    from concourse.dve_uop import DveOpSpec

    name = "SQDIFF_ANT"
    spec = Spec(
        body=sq(Src0 - C0),
        reference=lambda in0, in1, s0, s1, imm2: (in0.astype(np.float32) - s0) ** 2,
    )
    row = dve_ops._CUSTOM_DVE_ROW_BASE + len(dve_ops.OPS)
    assert row < 0x20
    dve_ops._SUB_OPCODE_FOR_NAME[name] = row
    # Pin the sha of our own lowering output (computed now, same process).
    shas = {}
    for ver in ("v3", "v4"):
        try:
            s = DveOpSpec(name=name, opcode=row, uops=lower(spec, ver=ver),
                          rd1_en=_has_src1(spec))
            shas[ver] = s.sha(ver)
        except Exception:
            pass
    op = dve_ops.DveOp(name, spec, subdim=False, uops_sha=shas)
    dve_ops.OPS.append(op)
    dve_ops.CUSTOM_DVE_SPECS[name] = spec
    _CACHE["sqdiff"] = op
    return op


def _bernstein_basis() -> np.ndarray:
    """bt [4, 128]: bt[j, p] = B_j(t_p), t = linspace(0,1,128) fp32."""
    t = np.linspace(0.0, 1.0, STEPS, dtype=np.float32).astype(np.float64)
    u = 1.0 - t
    bt = np.stack([u**3, 3 * t * u**2, 3 * t**2 * u, t**3])
    return bt.astype(np.float32)


def build_bass():
    import concourse.bass as bass
    import concourse.tile as tile
    from concourse import bacc, mybir

    sqdiff = _get_sqdiff()

    nc = bacc.Bacc("TRN2", target_bir_lowering=False, debug=False, num_devices=N_CORES)
    cvbt = nc.dram_tensor("cvbt", [4, 2 * N_CURVES + STEPS], mybir.dt.float32, kind="ExternalInput").ap()
    iaa = nc.dram_tensor("iaa", [STEPS, W], mybir.dt.float32, kind="ExternalInput").ap()
    out = nc.dram_tensor("out", [BROWS, RES], mybir.dt.float32, kind="ExternalOutput").ap()

    f32 = mybir.dt.float32
    f32r = mybir.dt.float32r
    Exp = mybir.ActivationFunctionType.Exp
    Square = mybir.ActivationFunctionType.Square
    NCV = 2 * N_CURVES

    with tile.TileContext(nc) as tc:
        with (
            tc.tile_pool(name="const", bufs=1) as cpool,
            tc.tile_pool(name="d", bufs=3) as dpool,
            tc.tile_pool(name="e", bufs=8) as epool,
            tc.tile_pool(name="res", bufs=1) as rpool,
            tc.tile_pool(name="psum", bufs=1, space="PSUM") as ppool,
            tc.tile_pool(name="psum_out", bufs=1, space="PSUM") as opool,
        ):
            cvbt_sb = cpool.tile([4, NCV + STEPS], f32)
            nc.sync.dma_start(out=cvbt_sb[:], in_=cvbt[:])
            iaa_sb = cpool.tile([STEPS, W], f32)
            nc.scalar.dma_start(out=iaa_sb[:], in_=iaa[:])

            # Dummy first ACT op with no DMA dependency: bacc inserts the
            # ~1.3us ACT_TABLE_LOAD before the first activation; anchoring it
            # here lets the load run at body start instead of behind a DMA wait.
            warm = cpool.tile([1, 2], f32)
            nc.vector.memset(warm[:], 0.0)
            nc.scalar.activation(warm[:, 1:2], warm[:, 0:1], Exp)

            # Bezier sampling: xy[p, 2i] = 512*x_i(t_p); xy[p,2i+1] = 512*y_i(t_p) - 64k
            psum_xy = ppool.tile([STEPS, NCV], f32)
            nc.tensor.matmul(
                psum_xy[:], lhsT=cvbt_sb[:, NCV:], rhs=cvbt_sb[:, 0:NCV],
                start=True, stop=True,
            )
            xy_sb = cpool.tile([STEPS, NCV], f32)
            nc.vector.tensor_copy(out=xy_sb[:], in_=psum_xy[:])

            psum_out = opool.tile([BROWS, RES], f32)

            for j in range(N_CURVES):
                d = dpool.tile([STEPS, W], f32)
                # y part: d[:, 512:576] = (r - (512*y_j - 64k))^2
                nc.vector._custom_dve(
                    sqdiff,
                    out=d[:, RES:W],
                    in0=iaa_sb[:, RES:W],
                    s0=xy_sb[:, 2 * j + 1 : 2 * j + 2],
                )
                # x part: d[:, 0:512] = (a - 512*x_j)^2
                if j == 0:
                    # balance: ACT does tile 0's x-part as Square(-a + x)
                    nc.scalar.activation(
                        d[:, 0:RES], iaa_sb[:, 0:RES], Square,
                        bias=xy_sb[:, 0:1], scale=-1.0,
                    )
                else:
                    nc.vector._custom_dve(
                        sqdiff,
                        out=d[:, 0:RES],
                        in0=iaa_sb[:, 0:RES],
                        s0=xy_sb[:, 2 * j : 2 * j + 1],
                    )
                e = epool.tile([STEPS, W], f32r)
                nc.scalar.activation(e[:], d[:], Exp, scale=EXP_SCALE)
                nc.tensor.matmul(
                    psum_out[:],
                    lhsT=e[:, RES:W],
                    rhs=e[:, 0:RES],
                    start=(j == 0),
                    stop=(j == N_CURVES - 1),
                )

            res_sb = rpool.tile([BROWS, RES], f32)
            nc.scalar.copy(out=res_sb[:], in_=psum_out[:])
            nc.sync.dma_start(out=out[:], in_=res_sb[:])

    nc.compile()
    return nc


def _make_inputs(curves: np.ndarray):
    """Per-core input maps."""
    bt = _bernstein_basis()
    iaa = np.zeros((STEPS, W), dtype=np.float32)
    iaa[:, :RES] = np.arange(RES, dtype=np.float32)[None, :]
    iaa[:, RES:] = np.arange(BROWS, dtype=np.float32)[None, :]

    xs = (RES * curves[:, :, 0]).astype(np.float32)  # [8,4] = 512*x control pts
    ys = (RES * curves[:, :, 1]).astype(np.float32)

    in_maps = []
    for k in range(N_CORES):
        cvbt = np.empty((4, 2 * N_CURVES + STEPS), dtype=np.float32)
        cvbt[:, 0 : 2 * N_CURVES : 2] = xs.T
        cvbt[:, 1 : 2 * N_CURVES : 2] = ys.T - np.float32(BROWS * k)
        cvbt[:, 2 * N_CURVES :] = bt
        in_maps.append({"cvbt": cvbt, "iaa": iaa})
    return in_maps


def kernel(curves: np.ndarray, trace: bool = False, tmpdir: str | None = None):
    _install_ntff_hook()
    from concourse.bass_utils import run_bass_kernel_spmd

    if "nc" not in _CACHE:
        _CACHE["nc"] = build_bass()
    nc = _CACHE["nc"]

    in_maps = _make_inputs(np.asarray(curves, dtype=np.float32))
    kw = {}
    if trace:
        import concourse.bass_utils as bu

        bu.upload_artifacts = lambda d: d  # no bucket in this container
        kw = {"trace": True, "tmpdir": tmpdir}
    res = run_bass_kernel_spmd(nc, in_maps, core_ids=list(range(N_CORES)), **kw)

    full = np.concatenate([res.results[k]["out"] for k in range(N_CORES)], axis=0)
    if trace:
        return full, res
    return full


# revision 13
# speedup vs baseline: 1.2517x; 1.0115x over previous
"""Bezier curve Gaussian rasterization on 8 Trainium2 NeuronCores.

Problem: curves [8,4,2] -> raster [512,512] where
    out[b,a] = sum_s Ey[b,s] * Ex[a,s]
    Ex[a,s] = exp(-5000*(x_s - a/512)^2),  x_s = cubic Bezier samples,
    T = 8 curves x 128 t-samples = 1024.

Strategy (no collectives -- their ~10us floor dwarfs this kernel):
shard OUTPUT ROWS b across the 8 cores. Core k computes
out[64k:64k+64, :] with the s-contraction (1024) done as 8 accumulating
float32r PE matmuls. Each core computes the full ExT (s on partitions,
8 tiles of [128, 512]) plus its own 64-wide Ey slice:
  d^2 via a custom DVE op sq(in0 - s0) (fuses sub+square, one op),
  exp via ACT, Bezier sampling via a tiny PE matmul against a baked
  Bernstein basis.

kernel(curves) -> np.ndarray [512,512] float32.
"""
import sys
import types

import numpy as np

RES = 512
STEPS = 128
N_CURVES = 8
N_CORES = 8
BROWS = RES // N_CORES  # 64 output rows per core
W = RES + BROWS  # 576 = per-tile width (x part | y part)
SIGMA = 0.01
# exp scale in pixel units: -(1/(2 sigma^2)) / RES^2
EXP_SCALE = -1.0 / (2.0 * SIGMA * SIGMA) / (RES * RES)

_CACHE = {}
GPSIMD_Y = False


def _install_ntff_hook():
    """Provide antenv.axon_hooks (missing in this image) so NTFF
    profiling via run_bass_kernel_spmd(trace=True) works."""
    try:
        import antenv
    except ImportError:
        return
    if "antenv.axon_hooks" in sys.modules:
        return
    mod = types.ModuleType("antenv.axon_hooks")
    _state = {"hook": None}
    mod.set_axon_ntff_profile_hook = lambda h: _state.__setitem__("hook", h)
    mod.get_axon_ntff_profile_hook = lambda: _state["hook"]
    sys.modules["antenv.axon_hooks"] = mod
    antenv.axon_hooks = mod
    try:
        from trn_agent_boot.trn_boot import _ntff_profile_via_ctypes

        hook = _ntff_profile_via_ctypes("/opt/axon/libaxon_pjrt.so")
        if hook is not None:
            mod.set_axon_ntff_profile_hook(hook)
    except Exception:
        pass


def _get_sqdiff():
    """Register (once) a custom DVE op: out = (in0 - s0)^2.

    Fuses the grid-minus-center subtract and the square into a single
    Vector-engine instruction; the table is generated per-NEFF so no
    firmware change is involved.
    """
    if "sqdiff" in _CACHE:
        return _CACHE["sqdiff"]
    from concourse import dve_ops
    from concourse.dve_spec import Spec, Src0, C0, sq, lower, _has_src1
    from concourse.dve_uop import DveOpSpec

    name = "SQDIFF_ANT"
    spec = Spec(
        body=sq(Src0 - C0),
        reference=lambda in0, in1, s0, s1, imm2: (in0.astype(np.float32) - s0) ** 2,
    )
    row = dve_ops._CUSTOM_DVE_ROW_BASE + len(dve_ops.OPS)
    assert row < 0x20
    dve_ops._SUB_OPCODE_FOR_NAME[name] = row
    # Pin the sha of our own lowering output (computed now, same process).
    shas = {}
    for ver in ("v3", "v4"):
        try:
            s = DveOpSpec(name=name, opcode=row, uops=lower(spec, ver=ver),
                          rd1_en=_has_src1(spec))
            shas[ver] = s.sha(ver)
        except Exception:
            pass
    op = dve_ops.DveOp(name, spec, subdim=False, uops_sha=shas)
    dve_ops.OPS.append(op)
    dve_ops.CUSTOM_DVE_SPECS[name] = spec
    _CACHE["sqdiff"] = op
    return op


def _bernstein_basis() -> np.ndarray:
    """bt [4, 128]: bt[j, p] = B_j(t_p), t = linspace(0,1,128) fp32."""
    t = np.linspace(0.0, 1.0, STEPS, dtype=np.float32).astype(np.float64)
    u = 1.0 - t
    bt = np.stack([u**3, 3 * t * u**2, 3 * t**2 * u, t**3])
    return bt.astype(np.float32)


def build_bass():
    import concourse.bass as bass
    import concourse.tile as tile
    from concourse import bacc, mybir

    sqdiff = _get_sqdiff()

    nc = bacc.Bacc("TRN2", target_bir_lowering=False, debug=False, num_devices=N_CORES)
    cvbt = nc.dram_tensor("cvbt", [4, 2 * N_CURVES + STEPS], mybir.dt.float32, kind="ExternalInput").ap()
    iaa = nc.dram_tensor("iaa", [STEPS, W], mybir.dt.float32, kind="ExternalInput").ap()
    out = nc.dram_tensor("out", [BROWS, RES], mybir.dt.float32, kind="ExternalOutput").ap()

    f32 = mybir.dt.float32
    f32r = mybir.dt.float32r
    Exp = mybir.ActivationFunctionType.Exp
    Square = mybir.ActivationFunctionType.Square
    NCV = 2 * N_CURVES

    # Pre-barrier input loads: issue the DMAs in the main block, before the
    # TileContext entry barrier -- the engines exit the NRT preamble ~1.3us
    # before the tile body starts, and the DMA-completion semaphore path adds
    # another ~2us, so issuing early hides nearly all of that latency.
    # Manual semaphores guard the first consumers; cleared post-exit so a
    # re-execution of the loaded NEFF starts from zero again.
    iaa_sb_t = nc.alloc_sbuf_tensor("iaa_sb_raw", [STEPS, W], f32)
    cvbt_sb_t = nc.alloc_sbuf_tensor("cvbt_sb_raw", [4, NCV + STEPS], f32)
    iaa_sem = nc.alloc_semaphore("iaa_in_sem")
    cvbt_sem = nc.alloc_semaphore("cvbt_in_sem")
    iaa_sb = iaa_sb_t.ap()
    cvbt_sb = cvbt_sb_t.ap()
    nc.scalar.dma_start(out=iaa_sb[:], in_=iaa[:]).then_inc(iaa_sem, 16)
    nc.sync.dma_start(out=cvbt_sb[:], in_=cvbt[:]).then_inc(cvbt_sem, 16)

    # The Tile scheduler's deadlock-check sim only simulates the tile block,
    # so waits on the pre-barrier DMA sems would falsely deadlock it (and the
    # consumer instructions' single wait slot is taken by Tile's own sems).
    # Instead: emit standalone wait_ge(sem, 0) EVSEMs -- trivially satisfied
    # in the scheduling sim -- and raise the threshold to 16 post-scheduling.
    deferred_waits = []

    def guard(engine, sem):
        deferred_waits.append((engine.wait_ge(sem, 0), sem))

    with tile.TileContext(nc) as tc:
        with (
            tc.tile_pool(name="const", bufs=1) as cpool,
            tc.tile_pool(name="d", bufs=3) as dpool,
            tc.tile_pool(name="e", bufs=8) as epool,
            tc.tile_pool(name="res", bufs=1) as rpool,
            tc.tile_pool(name="psum", bufs=1, space="PSUM") as ppool,
            tc.tile_pool(name="psum_out", bufs=1, space="PSUM") as opool,
        ):
            # Dummy first ACT op with no DMA dependency: bacc inserts the
            # ~1.3us ACT_TABLE_LOAD before the first activation; anchoring it
            # here lets the load run at body start instead of behind a DMA wait.
            warm = cpool.tile([1, 2], f32)
            nc.vector.memset(warm[:], 0.0)
            nc.scalar.activation(warm[:, 1:2], warm[:, 0:1], Exp)

            # Bezier sampling: xy[p, 2i] = 512*x_i(t_p); xy[p,2i+1] = 512*y_i(t_p) - 64k
            psum_xy = ppool.tile([STEPS, NCV], f32)
            guard(nc.tensor, cvbt_sem)
            nc.tensor.matmul(
                psum_xy[:], lhsT=cvbt_sb[:, NCV:], rhs=cvbt_sb[:, 0:NCV],
                start=True, stop=True,
            )
            xy_sb = cpool.tile([STEPS, NCV], f32)
            nc.vector.tensor_copy(out=xy_sb[:], in_=psum_xy[:])

            psum_out = opool.tile([BROWS, RES], f32)

            yeng = nc.gpsimd if GPSIMD_Y else nc.vector
            for j in range(N_CURVES):
                d = dpool.tile([STEPS, W], f32)
                # y part: d[:, 512:576] = (r - (512*y_j - 64k))^2
                if GPSIMD_Y:
                    if j == 0:
                        guard(nc.gpsimd, iaa_sem)
                    nc.gpsimd.tensor_scalar_sub(
                        out=d[:, RES:W], in0=iaa_sb[:, RES:W],
                        scalar1=xy_sb[:, 2 * j + 1 : 2 * j + 2],
                    )
                    nc.gpsimd.tensor_mul(
                        out=d[:, RES:W], in0=d[:, RES:W], in1=d[:, RES:W],
                    )
                else:
                    if j == 0:
                        guard(nc.vector, iaa_sem)
                    nc.vector._custom_dve(
                        sqdiff,
                        out=d[:, RES:W],
                        in0=iaa_sb[:, RES:W],
                        s0=xy_sb[:, 2 * j + 1 : 2 * j + 2],
                    )
                # x part: d[:, 0:512] = (a - 512*x_j)^2
                if j == 0:
                    # balance: ACT does tile 0's x-part as Square(-a + x)
                    guard(nc.scalar, iaa_sem)
                    nc.scalar.activation(
                        d[:, 0:RES], iaa_sb[:, 0:RES], Square,
                        bias=xy_sb[:, 0:1], scale=-1.0,
                    )
                else:
                    nc.vector._custom_dve(
                        sqdiff,
                        out=d[:, 0:RES],
                        in0=iaa_sb[:, 0:RES],
                        s0=xy_sb[:, 2 * j : 2 * j + 1],
                    )
                e = epool.tile([STEPS, W], f32r)
                nc.scalar.activation(e[:], d[:], Exp, scale=EXP_SCALE)
                nc.tensor.matmul(
                    psum_out[:],
                    lhsT=e[:, RES:W],
                    rhs=e[:, 0:RES],
                    start=(j == 0),
                    stop=(j == N_CURVES - 1),
                )

            res_sb = rpool.tile([BROWS, RES], f32)
            nc.scalar.copy(out=res_sb[:], in_=psum_out[:])
            nc.sync.dma_start(out=out[:], in_=res_sb[:])

    for inst, sem in deferred_waits:
        for wt in inst.ins.sync_info.on_wait:
            if wt.id == sem.num:
                wt.wait_value = 16

    # After the tile exit barriers: reset the manual input sems so a
    # re-execution of this loaded NEFF sees them at zero.
    nc.sync.sem_clear(iaa_sem)
    nc.sync.sem_clear(cvbt_sem)

    nc.compile()
    return nc


def _make_inputs(curves: np.ndarray):
    """Per-core input maps."""
    bt = _bernstein_basis()
    iaa = np.zeros((STEPS, W), dtype=np.float32)
    iaa[:, :RES] = np.arange(RES, dtype=np.float32)[None, :]
    iaa[:, RES:] = np.arange(BROWS, dtype=np.float32)[None, :]

    xs = (RES * curves[:, :, 0]).astype(np.float32)  # [8,4] = 512*x control pts
    ys = (RES * curves[:, :, 1]).astype(np.float32)

    in_maps = []
    for k in range(N_CORES):
        cvbt = np.empty((4, 2 * N_CURVES + STEPS), dtype=np.float32)
        cvbt[:, 0 : 2 * N_CURVES : 2] = xs.T
        cvbt[:, 1 : 2 * N_CURVES : 2] = ys.T - np.float32(BROWS * k)
        cvbt[:, 2 * N_CURVES :] = bt
        in_maps.append({"cvbt": cvbt, "iaa": iaa})
    return in_maps


def kernel(curves: np.ndarray, trace: bool = False, tmpdir: str | None = None):
    _install_ntff_hook()
    from concourse.bass_utils import run_bass_kernel_spmd

    if "nc" not in _CACHE:
        _CACHE["nc"] = build_bass()
    nc = _CACHE["nc"]

    in_maps = _make_inputs(np.asarray(curves, dtype=np.float32))
    kw = {}
    if trace:
        import concourse.bass_utils as bu

        bu.upload_artifacts = lambda d: d  # no bucket in this container
        kw = {"trace": True, "tmpdir": tmpdir}
    res = run_bass_kernel_spmd(nc, in_maps, core_ids=list(range(N_CORES)), **kw)

    full = np.concatenate([res.results[k]["out"] for k in range(N_CORES)], axis=0)
    if trace:
        return full, res
    return full
